# revision 37
# baseline (speedup 1.0000x reference)
"""Trainium2 Bass kernel for the Aligner module.

Computes, per batch b:
    g = sigmoid(conv2(relu(conv1(relu(x)))))          # [T] monotone gate
    ends = cumsum(g * valid_mask)                     # [T]
    centers = ends - 0.5*g ; aligned_len = ends[-1]
    w = softmax_t(-(centers[t]-pos[l])^2 / 10) masked # [L, T]
    out = w @ x^T                                     # [L, C]

Data-parallel over 8 NeuronCores: 4 batch elements per core, weights
replicated.  All matmuls run in bf16 (validated: output rel err ~2.4e-3
vs the fp32 reference, gate 2e-2).

Schedule notes (driven by neuron-profile traces):
  - host supplies relu(x) in [C,T] layout and x in [T,C] layout, both
    bf16 — no on-device relu / transpose of the big tensor is needed.
  - conv1 runs stationary-major (tt inner) with 4 PSUM banks so the PE
    stream has no stalls; conv2 for a batch is deferred until all its
    h tiles are drained, so it never blocks the PE behind a PSUM read.
  - the attention weights are built directly in T-major layout
    ([t_chunk=128, L]), so the output matmul needs no transposes, and
    the softmax denominator comes from a ones-column matmul that
    reuses the already-loaded stationary.
  - the cumsum phase is split into two batch-groups so the attention
    matmuls of batches 0-1 keep the PE busy while the cumsum of
    batches 2-3 runs on the vector engine (keeps HAM warm).
"""

import os
import sys

import numpy as np

B, C, T = 32, 512, 2048
L = 80
SIGMA2 = 10.0
NCORES = 8
BPC = B // NCORES  # batch elements per core

_cache = {}
LAST_RESULTS = None  # BassKernelResults of the most recent run (for profiling)


def _concourse():
    if "mods" in _cache:
        return _cache["mods"]
    if "/opt/trn_rl_repo" not in sys.path:
        sys.path.insert(0, "/opt/trn_rl_repo")
    import concourse.bass as bass
    import concourse.bacc as bacc
    import concourse.tile as tile
    from concourse import mybir
    from concourse import bass_utils

    _cache["mods"] = (bass, bacc, tile, mybir, bass_utils)
    return _cache["mods"]


def _build():
    """Build + compile the per-core Bass graph (cached)."""
    if "nc" in _cache:
        return _cache["nc"]
    bass, bacc, tile, mybir, _ = _concourse()
    from contextlib import ExitStack

    dt = mybir.dt
    f32, bf16 = dt.float32, dt.bfloat16
    Alu = mybir.AluOpType
    Act = mybir.ActivationFunctionType

    nc = bacc.Bacc("TRN2", target_bir_lowering=False)

    f8 = dt.float8e4
    # relu(x) and conv1 weights arrive fp8, packed for DoubleRow matmuls:
    # contraction index c = kg*256 + i*128 + p  ->  [kg, p, i, ...]
    xbf = nc.declare_dram_parameter("xbf", [BPC, 2, 128, 2, T], f8, isOutput=False)
    xt = nc.declare_dram_parameter("xt", [BPC, T, C], bf16, isOutput=False)
    w1t = nc.declare_dram_parameter("w1t", [2, 128, 2, C], f8, isOutput=False)
    w2 = nc.declare_dram_parameter("w2", [128, 4], bf16, isOutput=False)  # col oc
    b1 = nc.declare_dram_parameter("b1", [128, 4], f32, isOutput=False)  # col oc
    b2r = nc.declare_dram_parameter("b2r", [BPC, 1], f32, isOutput=False)
    pos = nc.declare_dram_parameter("pos", [L, 1], f32, isOutput=False)
    lenf = nc.declare_dram_parameter("lenf", [BPC, 1], f32, isOutput=False)
    out = nc.declare_dram_parameter("out", [BPC, L, C], f32, isOutput=True)
    olen = nc.declare_dram_parameter("olen", [BPC, 1], f32, isOutput=True)
    g_dram = nc.dram_tensor("g_scratch", [BPC, T], f32)
    ct_dram = nc.dram_tensor("ct_scratch", [BPC, 128, 16], f32)


    with ExitStack() as ctx:
        tc = ctx.enter_context(tile.TileContext(nc))
        singles = ctx.enter_context(tc.tile_pool(name="singles", bufs=1))
        xpool = ctx.enter_context(tc.tile_pool(name="xpool", bufs=16))
        xtpool = ctx.enter_context(tc.tile_pool(name="xtpool", bufs=26))
        hpool = ctx.enter_context(tc.tile_pool(name="hpool", bufs=20))
        gtp = ctx.enter_context(tc.tile_pool(name="gtp", bufs=3))
        ctpool = ctx.enter_context(tc.tile_pool(name="ctpool", bufs=4))
        dpool = ctx.enter_context(tc.tile_pool(name="dpool", bufs=2))
        ewpool = ctx.enter_context(tc.tile_pool(name="ewpool", bufs=2))
        cepool = ctx.enter_context(tc.tile_pool(name="cepool", bufs=2))
        opool = ctx.enter_context(tc.tile_pool(name="opool", bufs=3))
        smalls = ctx.enter_context(tc.tile_pool(name="smalls", bufs=4))
        growp = ctx.enter_context(tc.tile_pool(name="growp", bufs=1))
        psum_h = ctx.enter_context(tc.tile_pool(name="psum_h", bufs=4, space="PSUM"))
        psum_g = ctx.enter_context(tc.tile_pool(name="psum_g", bufs=2, space="PSUM"))
        psum_o = ctx.enter_context(tc.tile_pool(name="psum_o", bufs=1, space="PSUM"))
        psum_s = ctx.enter_context(tc.tile_pool(name="psum_s", bufs=1, space="PSUM"))

        # --- replicated weights / constants ---
        w1t_sb = []
        for kg in range(2):
            t_ = singles.tile([128, 2, C], f8, tag=f"w1t{kg}")
            nc.sync.dma_start(out=t_, in_=w1t[kg])
            w1t_sb.append(t_)
        w2_sb = singles.tile([128, 4], bf16, tag="w2")
        nc.sync.dma_start(out=w2_sb, in_=w2[:, :])
        b1_sb = singles.tile([128, 4], f32, tag="b1")
        nc.sync.dma_start(out=b1_sb, in_=b1[:, :])
        b2_sb = singles.tile([BPC, 1], f32, tag="b2")
        nc.sync.dma_start(out=b2_sb, in_=b2r[:, :])
        # pos values replicated on every partition and every t-chunk:
        # posK[p, k, l] = pos[l]
        posK = singles.tile([128, 16, L], f32, tag="posk")
        pos_flat = pos[:, :]
        for k in range(16):
            nc.sync.dma_start(
                out=posK[:, k, :],
                in_=bass.AP(tensor=pos_flat.tensor, offset=pos_flat.offset, ap=[[0, 128], [1, L]]),
            )
        ones_col = singles.tile([128, 1], bf16, tag="ones")
        nc.vector.memset(ones_col, 1.0)
        # The gate/cumsum phase runs in T-major layout ([128, 16] tiles,
        # t = k*128 + p).  Constants for it:
        ones_colf = singles.tile([128, 1], f32, tag="onesf")
        nc.vector.memset(ones_colf, 1.0)
        ones_row = singles.tile([1, 128], f32, tag="onesr")
        nc.vector.memset(ones_row, 1.0)
        # lower-triangular ones (q <= p) for the intra-chunk prefix matmul:
        # iota(q,p) = p - q, then compare >= 0
        triu = singles.tile([128, 128], f32, tag="triu")
        nc.gpsimd.iota(
            triu,
            pattern=[[1, 128]],
            base=0,
            channel_multiplier=-1,
            allow_small_or_imprecise_dtypes=True,
        )
        nc.vector.tensor_scalar(triu, triu, 0.0, None, Alu.is_ge)
        iota_tm = singles.tile([128, 16], f32, tag="iotatm")
        nc.gpsimd.iota(
            iota_tm,
            pattern=[[128, 16]],
            base=0,
            channel_multiplier=1,  # value = p + 128*k = t
            allow_small_or_imprecise_dtypes=True,
        )
        # per-batch sequence length replicated on every partition
        lenf_bc = singles.tile([128, BPC], f32, tag="lenbc")
        lf = lenf[:, :]
        nc.sync.dma_start(
            out=lenf_bc,
            in_=bass.AP(tensor=lf.tensor, offset=lf.offset, ap=[[0, 128], [1, BPC]]),
        )
        # (t >= len_b) * 1e6 in T-major, per batch: pushes masked-out
        # positions far from every out position so exp underflows to 0
        bigm_b = []
        for _b in range(BPC):
            bm = singles.tile([128, 16], f32, tag=f"bigm{_b}")
            nc.vector.tensor_scalar(
                bm, iota_tm, lenf_bc[:, _b : _b + 1], 1e6, Alu.is_ge, Alu.mult
            )
            bigm_b.append(bm)

        def conv_batch(b):
            """conv1 + conv2 + sigmoid for batch element b -> g_sb row b."""
            # two half-T tiles per DoubleRow k-group: a conv1 matmul only
            # ever reads within one half, and the halves land on different
            # DMA queues in parallel (halves the time to the first matmul)
            rx = []
            for kg in range(2):
                ha = xpool.tile([128, 2, T // 2], f8, tag="xbf", name=f"xa{b}_{kg}")
                nc.sync.dma_start(out=ha, in_=xbf[b, kg, :, :, 0 : T // 2])
                hb_ = xpool.tile([128, 2, T // 2], f8, tag="xbf", name=f"xc{b}_{kg}")
                nc.sync.dma_start(out=hb_, in_=xbf[b, kg, :, :, T // 2 : T])
                rx.append((ha, hb_))
            hs = {}
            for oc in range(4):
                os_ = slice(oc * 128, (oc + 1) * 128)
                phs = [psum_h.tile([128, 512], f32, tag="ph", name=f"ph{oc}_{i}") for i in range(4)]
                # stationary-major fp8 DoubleRow: each w1t[kg, oc] block is
                # loaded once and streamed over the 4 t-tiles; the PE runs
                # 2 contraction rows per cell (0.5 cycles per output row)
                for kg in range(2):
                    for tt in range(4):
                        half = rx[kg][tt // 2]
                        hs_ = slice((tt % 2) * 512, (tt % 2 + 1) * 512)
                        nc.tensor.matmul(
                            phs[tt],
                            w1t_sb[kg][:, :, os_],
                            half[:, :, hs_],
                            start=(kg == 0),
                            stop=(kg == 1),
                            perf_mode=mybir.MatmulPerfMode.DoubleRow,
                        )
                for tt in range(4):
                    h = hpool.tile([128, 512], bf16, tag="h")
                    # h = max(psum + b1, 0) in bf16 (DVE keeps ACT free for
                    # the square/exp chain and avoids LUT swaps)
                    nc.vector.tensor_scalar(
                        h, phs[tt], b1_sb[:, oc : oc + 1], 0.0, Alu.add, Alu.max
                    )
                    hs[(oc, tt)] = h
            # conv2: all h tiles are ready by now, so the PE never stalls
            g_row = growp.tile([1, T], f32, tag="grow")
            for tt in range(4):
                pg = psum_g.tile([1, 512], f32, tag="pg")
                for oc in range(4):
                    nc.tensor.matmul(
                        pg,
                        w2_sb[:, oc : oc + 1],
                        hs[(oc, tt)],
                        start=(oc == 0),
                        stop=(oc == 3),
                    )
                nc.scalar.activation(
                    g_row[:, tt * 512 : (tt + 1) * 512],
                    pg,
                    Act.Sigmoid,
                    bias=b2_sb[0:1, :],
                    scale=1.0,
                )
            # bounce the gate row through DRAM: phase C re-reads it in
            # T-major [128, 16] (SBUF sources cannot re-tile free->partition)
            nc.sync.dma_start(out=g_dram[b : b + 1, :], in_=g_row)

        cts = {}

        def phase_c(b):
            """Gate mask, cumsum and NEGATED masked centers for batch b,
            all in T-major [128, 16] (t = k*128 + p).

            cumsum(t) = (intra-chunk prefix via a triangular-ones matmul)
                      + (exclusive prefix of the 16 chunk sums, broadcast
                         back across partitions with a K=1 matmul into the
                         same PSUM accumulator)
            Leaves ~1us of vector work; everything heavy is on the PE.
            """
            gT = gtp.tile([128, 16], f32, tag="gt")
            gr = g_dram[b : b + 1, :]
            nc.sync.dma_start(
                out=gT,
                in_=bass.AP(tensor=gr.tensor, offset=gr.offset, ap=[[1, 128], [128, 16]]),
            )
            gm_tm = gtp.tile([128, 16], f32, tag="gmt")
            # gm = (t < len_b) * g
            nc.vector.scalar_tensor_tensor(
                gm_tm, iota_tm, lenf_bc[:, b : b + 1], gT, Alu.is_lt, Alu.mult
            )
            # chunk sums -> [1, 16]
            s_ps = psum_s.tile([1, 16], f32, tag="ps", name=f"sps{b}")
            nc.tensor.matmul(s_ps, ones_colf, gm_tm, start=True, stop=True)
            s_sb = smalls.tile([1, 16], f32, tag="ssb")
            nc.vector.tensor_copy(s_sb, s_ps)
            incl = smalls.tile([1, 16], f32, tag="incl")
            nc.vector.tensor_tensor_scan(incl, s_sb, s_sb, 0.0, Alu.add, Alu.bypass)
            nc.sync.dma_start(out=olen[b : b + 1, :], in_=incl[:, 15:16])
            offs = smalls.tile([1, 16], f32, tag="offs")
            nc.vector.tensor_tensor(offs, incl, s_sb, Alu.subtract)
            # full cumsum: triangular prefix + broadcast chunk offsets
            cumT = psum_s.tile([128, 16], f32, tag="ps", name=f"cum{b}")
            nc.tensor.matmul(cumT, triu, gm_tm, start=True, stop=False)
            nc.tensor.matmul(cumT, ones_row, offs, start=False, stop=True)
            # negated masked centers: ct = 0.5*gm - cumsum - bigm, so the
            # attention distance is one ACT op: (pos + ct)^2 = (pos-centers)^2
            ct = ctpool.tile([128, 16], f32, tag="ct", name=f"ct{b}")
            nc.vector.scalar_tensor_tensor(
                ct, gm_tm, 0.5, cumT, Alu.mult, Alu.subtract
            )
            nc.vector.tensor_tensor(ct, ct, bigm_b[b], Alu.subtract)
            # expand to ce[p, k, l] = ct[p, k] with a zero-step read on the
            # vector engine, so the distance/exp chain runs as one wide op
            ce = cepool.tile([128, 16, L], f32, tag="ce", name=f"ce{b}")
            src_ap = bass.AP(
                tensor=ct.tensor,
                offset=ct.offset,
                ap=[list(ct.ap[0]), [1, 16], [0, L]],
            )
            nc.vector.tensor_copy(ce, src_ap)
            cts[b] = ce

        def phase_d(b, prev=None):
            """softmax attention + output matmul for batch element b.

            The attention weights are built directly in T-major layout
            ([t_chunk=128, L]) so the output matmul needs no transposes:
              ewT[t, l] = exp(-(centers[t] - pos[l])^2 / sigma2)
              out[l, c] = sum_t ewT[t, l] * xT[t, c]   (16 accumulating MMs)
              sums[l]   = sum_t ewT[t, l] * 1          (ones-column MMs)
            """
            xts = []
            for k2 in range(8):
                xt_t = xtpool.tile([128, 2, C], bf16, tag="xt")
                nc.sync.dma_start(
                    out=xt_t,
                    in_=xt[b, k2 * 256 : (k2 + 1) * 256, :].rearrange(
                        "(a p) c -> p a c", p=128
                    ),
                )
                xts.append(xt_t)
            if prev is not None:
                # previous batch's output store, emitted here so it sits
                # behind this batch's prefetches in the in-order sync queue
                pb, pob = prev
                nc.sync.dma_start(out=out[pb], in_=pob)
            ce = cts[b]
            po = psum_o.tile([L, C], f32, tag="po")
            ps = psum_s.tile([L, 1], f32, tag="ps")
            # distance and gaussian over all 16 chunks in one wide op each
            dt_ = dpool.tile([128, 16, L], f32, tag="dt", name=f"dt{b}")
            nc.vector.tensor_tensor(dt_, posK, ce, Alu.add)  # pos - centers
            nc.scalar.activation(dt_, dt_, Act.Square, bias=0.0, scale=1.0)
            ew_all = ewpool.tile([128, 16, L], bf16, tag="ewt", name=f"ew{b}")
            nc.scalar.activation(ew_all, dt_, Act.Exp, bias=0.0, scale=-1.0 / SIGMA2)
            for k in range(16):
                ewt = ew_all[:, k, :]
                nc.tensor.matmul(
                    po, ewt, xts[k // 2][:, k % 2, :], start=(k == 0), stop=(k == 15)
                )
                nc.tensor.matmul(ps, ewt, ones_col, start=(k == 0), stop=(k == 15))
            rsum = smalls.tile([L, 1], f32, tag="rsum")
            nc.vector.reciprocal(rsum, ps)
            ob = opool.tile([L, C], f32, tag="ob")
            nc.vector.tensor_scalar_mul(ob, po, rsum)
            return ob

        # conv for all batches first (dense PE stream), then the cumsum of
        # batches 0-1 overlaps the tail of conv 2-3 on the vector engine,
        # and the attention matmuls of 0-1 keep the PE busy while the
        # cumsum of 2-3 runs.
        # each phase_c(b) is emitted one conv batch late so its T-major
        # gather DMA has landed before the PE reaches its tiny matmuls
        conv_batch(0)
        conv_batch(1)
        phase_c(0)
        conv_batch(2)
        phase_c(1)
        conv_batch(3)
        phase_c(2)
        ob0 = phase_d(0)
        ob1 = phase_d(1, prev=(0, ob0))
        phase_c(3)
        ob2 = phase_d(2, prev=(1, ob1))
        ob3 = phase_d(3, prev=(2, ob2))
        # final store split across two DMA queues to shorten the tail
        nc.sync.dma_start(out=out[3, 0 : L // 2, :], in_=ob3[0 : L // 2, :])
        nc.sync.dma_start(out=out[3, L // 2 : L, :], in_=ob3[L // 2 : L, :])

    nc.compile()
    _cache["nc"] = nc
    return nc


def _in_maps(x, len_fea, conv1_w, conv1_b, conv2_w, conv2_b):
    import ml_dtypes

    bf16 = ml_dtypes.bfloat16
    f8 = ml_dtypes.float8_e4m3fn
    x = np.asarray(x, np.float32)
    x_bf = x.astype(bf16)  # [B, C, T] (feeds the output einsum, transposed)
    xt_bf = np.ascontiguousarray(x_bf.transpose(0, 2, 1))  # [B, T, C]
    # relu folded into the host prep; fp8, packed for DoubleRow:
    # [b, kg, p, i, t] with c = kg*256 + i*128 + p
    rx_bf = np.ascontiguousarray(
        np.maximum(x, 0).reshape(B, 2, 2, 128, T).transpose(0, 1, 3, 2, 4)
    ).astype(f8)
    w1t_h = np.ascontiguousarray(
        np.asarray(conv1_w, np.float32).T.reshape(2, 2, 128, C).transpose(0, 2, 1, 3)
    ).astype(f8)
    w2_h = np.ascontiguousarray(np.asarray(conv2_w, np.float32).reshape(4, 128).T).astype(bf16)
    b1_h = np.ascontiguousarray(np.asarray(conv1_b, np.float32).reshape(4, 128).T)
    b2_h = np.full((BPC, 1), float(np.asarray(conv2_b)), np.float32)
    pos_h = (0.5 + np.arange(L, dtype=np.float32)).reshape(L, 1)
    lenf_h = np.asarray(len_fea).astype(np.float32).reshape(B, 1)
    maps = []
    for i in range(NCORES):
        s = slice(i * BPC, (i + 1) * BPC)
        maps.append(
            {
                "xbf": np.ascontiguousarray(rx_bf[s]),
                "xt": np.ascontiguousarray(xt_bf[s]),
                "w1t": w1t_h,
                "w2": w2_h,
                "b1": b1_h,
                "b2r": b2_h,
                "pos": pos_h,
                "lenf": np.ascontiguousarray(lenf_h[s]),
            }
        )
    return maps


def _install_ntff_shim():
    """Provide antenv.axon_hooks (NTFF profile hook) when the image's
    antenv package lacks it, driving profiling via ctypes into
    libaxon_pjrt.so.  Needed only for BASS_TRACE=1 profiling runs."""
    import types
    import ctypes
    import contextlib

    try:
        from antenv.axon_hooks import get_axon_ntff_profile_hook  # noqa: F401

        return
    except ImportError:
        pass

    holder = {"h": None}
    mod = types.ModuleType("antenv.axon_hooks")
    mod.set_axon_ntff_profile_hook = lambda h: holder.__setitem__("h", h)
    mod.get_axon_ntff_profile_hook = lambda: holder["h"]
    sys.modules["antenv.axon_hooks"] = mod

    so_path = "/opt/axon/libaxon_pjrt.so"
    if not os.path.exists(so_path):
        return
    lib = ctypes.CDLL(so_path)
    if not hasattr(lib, "axon_start_nrt_profile"):
        return
    lib.axon_start_nrt_profile.argtypes = [
        ctypes.POINTER(ctypes.c_int64),
        ctypes.c_size_t,
    ]
    lib.axon_start_nrt_profile.restype = ctypes.c_int64
    lib.axon_stop_nrt_profile.argtypes = [ctypes.c_char_p]
    lib.axon_stop_nrt_profile.restype = ctypes.c_int64

    @contextlib.contextmanager
    def _hook(output_dir, device_ids):
        import jax

        jax.devices()
        if device_ids:
            ids = (ctypes.c_int64 * len(device_ids))(*device_ids)
            rc = lib.axon_start_nrt_profile(ids, len(device_ids))
        else:
            rc = lib.axon_start_nrt_profile(None, 0)
        if rc != 0:
            raise RuntimeError(f"axon_start_nrt_profile rc={rc}")
        try:
            yield
        finally:
            n = lib.axon_stop_nrt_profile(str(output_dir).encode())
            print(f"ntff profile: {n} file(s) written to {output_dir}", file=sys.stderr)

    mod.set_axon_ntff_profile_hook(_hook)


def kernel(x, len_fea, conv1_w, conv1_b, conv2_w, conv2_b):
    global LAST_RESULTS
    *_, bass_utils = _concourse()
    _install_ntff_shim()
    nc = _build()
    maps = _in_maps(x, len_fea, conv1_w, conv1_b, conv2_w, conv2_b)
    res = bass_utils.run_bass_kernel_spmd(nc, maps, core_ids=list(range(NCORES)))
    LAST_RESULTS = res
    feats = np.concatenate([np.asarray(r["out"]) for r in res.results], axis=0)
    lens = np.concatenate([np.asarray(r["olen"])[:, 0] for r in res.results], axis=0)
    return feats.astype(np.float32), lens.astype(np.float32)


# revision 38
# speedup vs baseline: 1.4263x; 1.4263x over previous
"""Trainium2 Bass kernel for the Aligner module.

Computes, per batch b:
    g = sigmoid(conv2(relu(conv1(relu(x)))))          # [T] monotone gate
    ends = cumsum(g * valid_mask)                     # [T]
    centers = ends - 0.5*g ; aligned_len = ends[-1]
    w = softmax_t(-(centers[t]-pos[l])^2 / 10) masked # [L, T]
    out = w @ x^T                                     # [L, C]

Data-parallel over 8 NeuronCores: 4 batch elements per core, weights
replicated.  All matmuls run in bf16 (validated: output rel err ~2.4e-3
vs the fp32 reference, gate 2e-2).

Schedule notes (driven by neuron-profile traces):
  - host supplies relu(x) in [C,T] layout and x in [T,C] layout, both
    bf16 — no on-device relu / transpose of the big tensor is needed.
  - conv1 runs stationary-major (tt inner) with 4 PSUM banks so the PE
    stream has no stalls; conv2 for a batch is deferred until all its
    h tiles are drained, so it never blocks the PE behind a PSUM read.
  - the attention weights are built directly in T-major layout
    ([t_chunk=128, L]), so the output matmul needs no transposes, and
    the softmax denominator comes from a ones-column matmul that
    reuses the already-loaded stationary.
  - the cumsum phase is split into two batch-groups so the attention
    matmuls of batches 0-1 keep the PE busy while the cumsum of
    batches 2-3 runs on the vector engine (keeps HAM warm).
"""

import os
import sys

import numpy as np

B, C, T = 32, 512, 2048
L = 80
SIGMA2 = 10.0
NCORES = 8
BPC = B // NCORES  # batch elements per core

_cache = {}
LAST_RESULTS = None  # BassKernelResults of the most recent run (for profiling)


def _concourse():
    if "mods" in _cache:
        return _cache["mods"]
    if "/opt/trn_rl_repo" not in sys.path:
        sys.path.insert(0, "/opt/trn_rl_repo")
    import concourse.bass as bass
    import concourse.bacc as bacc
    import concourse.tile as tile
    from concourse import mybir
    from concourse import bass_utils

    _cache["mods"] = (bass, bacc, tile, mybir, bass_utils)
    return _cache["mods"]


def _build():
    """Build + compile the per-core Bass graph (cached)."""
    if "nc" in _cache:
        return _cache["nc"]
    bass, bacc, tile, mybir, _ = _concourse()
    from contextlib import ExitStack

    dt = mybir.dt
    f32, bf16 = dt.float32, dt.bfloat16
    Alu = mybir.AluOpType
    Act = mybir.ActivationFunctionType

    nc = bacc.Bacc("TRN2", target_bir_lowering=False)

    f8 = dt.float8e4
    # relu(x) and conv1 weights arrive fp8, packed for DoubleRow matmuls:
    # contraction index c = kg*256 + i*128 + p  ->  [kg, p, i, ...]
    xbf = nc.declare_dram_parameter("xbf", [BPC, 2, 128, 2, T], f8, isOutput=False)
    xt = nc.declare_dram_parameter("xt", [BPC, T, C], bf16, isOutput=False)
    w1t = nc.declare_dram_parameter("w1t", [2, 128, 2, C], f8, isOutput=False)
    w2 = nc.declare_dram_parameter("w2", [128, 4], bf16, isOutput=False)  # col oc
    b1 = nc.declare_dram_parameter("b1", [128, 4], f32, isOutput=False)  # col oc
    b2r = nc.declare_dram_parameter("b2r", [BPC, 1], f32, isOutput=False)
    pos = nc.declare_dram_parameter("pos", [L, 1], f32, isOutput=False)
    lenf = nc.declare_dram_parameter("lenf", [BPC, 1], f32, isOutput=False)
    out = nc.declare_dram_parameter("out", [BPC, L, C], f32, isOutput=True)
    olen = nc.declare_dram_parameter("olen", [BPC, 1], f32, isOutput=True)
    g_dram = nc.dram_tensor("g_scratch", [BPC, T], f32)
    ct_dram = nc.dram_tensor("ct_scratch", [BPC, 128, 16], f32)


    with ExitStack() as ctx:
        tc = ctx.enter_context(tile.TileContext(nc))
        singles = ctx.enter_context(tc.tile_pool(name="singles", bufs=1))
        xpool = ctx.enter_context(tc.tile_pool(name="xpool", bufs=16))
        xtpool = ctx.enter_context(tc.tile_pool(name="xtpool", bufs=26))
        hpool = ctx.enter_context(tc.tile_pool(name="hpool", bufs=20))
        gtp = ctx.enter_context(tc.tile_pool(name="gtp", bufs=3))
        ctpool = ctx.enter_context(tc.tile_pool(name="ctpool", bufs=4))
        dpool = ctx.enter_context(tc.tile_pool(name="dpool", bufs=2))
        ewpool = ctx.enter_context(tc.tile_pool(name="ewpool", bufs=2))
        cepool = ctx.enter_context(tc.tile_pool(name="cepool", bufs=2))
        opool = ctx.enter_context(tc.tile_pool(name="opool", bufs=3))
        smalls = ctx.enter_context(tc.tile_pool(name="smalls", bufs=4))
        growp = ctx.enter_context(tc.tile_pool(name="growp", bufs=1))
        psum_h = ctx.enter_context(tc.tile_pool(name="psum_h", bufs=4, space="PSUM"))
        psum_g = ctx.enter_context(tc.tile_pool(name="psum_g", bufs=2, space="PSUM"))
        psum_o = ctx.enter_context(tc.tile_pool(name="psum_o", bufs=1, space="PSUM"))
        psum_s = ctx.enter_context(tc.tile_pool(name="psum_s", bufs=1, space="PSUM"))

        # --- replicated weights / constants ---
        w1t_sb = []
        for kg in range(2):
            t_ = singles.tile([128, 2, C], f8, tag=f"w1t{kg}")
            nc.sync.dma_start(out=t_, in_=w1t[kg])
            w1t_sb.append(t_)
        w2_sb = singles.tile([128, 4], bf16, tag="w2")
        nc.sync.dma_start(out=w2_sb, in_=w2[:, :])
        b1_sb = singles.tile([128, 4], f32, tag="b1")
        nc.sync.dma_start(out=b1_sb, in_=b1[:, :])
        b2_sb = singles.tile([BPC, 1], f32, tag="b2")
        nc.sync.dma_start(out=b2_sb, in_=b2r[:, :])
        # pos values replicated on every partition and every t-chunk:
        # posK[p, k, l] = pos[l] (one broadcast DMA + one zero-step copy)
        pos_bc = singles.tile([128, L], f32, tag="posbc")
        pos_flat = pos[:, :]
        nc.sync.dma_start(
            out=pos_bc,
            in_=bass.AP(tensor=pos_flat.tensor, offset=pos_flat.offset, ap=[[0, 128], [1, L]]),
        )
        posK = singles.tile([128, 16, L], f32, tag="posk")
        nc.vector.tensor_copy(
            posK,
            bass.AP(tensor=pos_bc.tensor, offset=pos_bc.offset, ap=[list(pos_bc.ap[0]), [0, 16], [1, L]]),
        )
        ones_col = singles.tile([128, 1], bf16, tag="ones")
        nc.vector.memset(ones_col, 1.0)
        # The gate/cumsum phase runs in T-major layout ([128, 16] tiles,
        # t = k*128 + p).  Constants for it:
        ones_colf = singles.tile([128, 1], f32, tag="onesf")
        nc.vector.memset(ones_colf, 1.0)
        ones_row = singles.tile([1, 128], f32, tag="onesr")
        nc.vector.memset(ones_row, 1.0)
        # lower-triangular ones (q <= p) for the intra-chunk prefix matmul:
        # iota(q,p) = p - q, then compare >= 0
        triu = singles.tile([128, 128], f32, tag="triu")
        nc.gpsimd.iota(
            triu,
            pattern=[[1, 128]],
            base=0,
            channel_multiplier=-1,
            allow_small_or_imprecise_dtypes=True,
        )
        nc.vector.tensor_scalar(triu, triu, 0.0, None, Alu.is_ge)
        iota_tm = singles.tile([128, 16], f32, tag="iotatm")
        nc.gpsimd.iota(
            iota_tm,
            pattern=[[128, 16]],
            base=0,
            channel_multiplier=1,  # value = p + 128*k = t
            allow_small_or_imprecise_dtypes=True,
        )
        # per-batch sequence length replicated on every partition
        lenf_bc = singles.tile([128, BPC], f32, tag="lenbc")
        lf = lenf[:, :]
        nc.sync.dma_start(
            out=lenf_bc,
            in_=bass.AP(tensor=lf.tensor, offset=lf.offset, ap=[[0, 128], [1, BPC]]),
        )
        # (t >= len_b) * 1e6 in T-major, per batch: pushes masked-out
        # positions far from every out position so exp underflows to 0
        bigm_b = []
        for _b in range(BPC):
            bm = singles.tile([128, 16], f32, tag=f"bigm{_b}")
            nc.vector.tensor_scalar(
                bm, iota_tm, lenf_bc[:, _b : _b + 1], 1e6, Alu.is_ge, Alu.mult
            )
            bigm_b.append(bm)

        def conv_batch(b):
            """conv1 + conv2 + sigmoid for batch element b -> g_sb row b."""
            # two half-T tiles per DoubleRow k-group: a conv1 matmul only
            # ever reads within one half, and the halves land on different
            # DMA queues in parallel (halves the time to the first matmul)
            rx = []
            for kg in range(2):
                ha = xpool.tile([128, 2, T // 2], f8, tag="xbf", name=f"xa{b}_{kg}")
                nc.sync.dma_start(out=ha, in_=xbf[b, kg, :, :, 0 : T // 2])
                hb_ = xpool.tile([128, 2, T // 2], f8, tag="xbf", name=f"xc{b}_{kg}")
                nc.sync.dma_start(out=hb_, in_=xbf[b, kg, :, :, T // 2 : T])
                rx.append((ha, hb_))
            hs = {}
            for oc in range(4):
                os_ = slice(oc * 128, (oc + 1) * 128)
                phs = [psum_h.tile([128, 512], f32, tag="ph", name=f"ph{oc}_{i}") for i in range(4)]
                # stationary-major fp8 DoubleRow: each w1t[kg, oc] block is
                # loaded once and streamed over the 4 t-tiles; the PE runs
                # 2 contraction rows per cell (0.5 cycles per output row)
                for kg in range(2):
                    for tt in range(4):
                        half = rx[kg][tt // 2]
                        hs_ = slice((tt % 2) * 512, (tt % 2 + 1) * 512)
                        nc.tensor.matmul(
                            phs[tt],
                            w1t_sb[kg][:, :, os_],
                            half[:, :, hs_],
                            start=(kg == 0),
                            stop=(kg == 1),
                            perf_mode=mybir.MatmulPerfMode.DoubleRow,
                        )
                for tt in range(4):
                    h = hpool.tile([128, 512], bf16, tag="h")
                    # h = max(psum + b1, 0) in bf16 (DVE keeps ACT free for
                    # the square/exp chain and avoids LUT swaps)
                    nc.vector.tensor_scalar(
                        h, phs[tt], b1_sb[:, oc : oc + 1], 0.0, Alu.add, Alu.max
                    )
                    hs[(oc, tt)] = h
            # conv2: all h tiles are ready by now, so the PE never stalls
            g_row = growp.tile([1, T], f32, tag="grow")
            for tt in range(4):
                pg = psum_g.tile([1, 512], f32, tag="pg")
                for oc in range(4):
                    nc.tensor.matmul(
                        pg,
                        w2_sb[:, oc : oc + 1],
                        hs[(oc, tt)],
                        start=(oc == 0),
                        stop=(oc == 3),
                    )
                nc.scalar.activation(
                    g_row[:, tt * 512 : (tt + 1) * 512],
                    pg,
                    Act.Sigmoid,
                    bias=b2_sb[0:1, :],
                    scale=1.0,
                )
            # bounce the gate row through DRAM: phase C re-reads it in
            # T-major [128, 16] (SBUF sources cannot re-tile free->partition)
            nc.sync.dma_start(out=g_dram[b : b + 1, :], in_=g_row)

        cts = {}

        def phase_c(b):
            """Gate mask, cumsum and NEGATED masked centers for batch b,
            all in T-major [128, 16] (t = k*128 + p).

            cumsum(t) = (intra-chunk prefix via a triangular-ones matmul)
                      + (exclusive prefix of the 16 chunk sums, broadcast
                         back across partitions with a K=1 matmul into the
                         same PSUM accumulator)
            Leaves ~1us of vector work; everything heavy is on the PE.
            """
            gT = gtp.tile([128, 16], f32, tag="gt")
            gr = g_dram[b : b + 1, :]
            nc.sync.dma_start(
                out=gT,
                in_=bass.AP(tensor=gr.tensor, offset=gr.offset, ap=[[1, 128], [128, 16]]),
            )
            gm_tm = gtp.tile([128, 16], f32, tag="gmt")
            # gm = (t < len_b) * g
            nc.vector.scalar_tensor_tensor(
                gm_tm, iota_tm, lenf_bc[:, b : b + 1], gT, Alu.is_lt, Alu.mult
            )
            # chunk sums -> [1, 16]
            s_ps = psum_s.tile([1, 16], f32, tag="ps", name=f"sps{b}")
            nc.tensor.matmul(s_ps, ones_colf, gm_tm, start=True, stop=True)
            s_sb = smalls.tile([1, 16], f32, tag="ssb")
            nc.vector.tensor_copy(s_sb, s_ps)
            incl = smalls.tile([1, 16], f32, tag="incl")
            nc.vector.tensor_tensor_scan(incl, s_sb, s_sb, 0.0, Alu.add, Alu.bypass)
            nc.sync.dma_start(out=olen[b : b + 1, :], in_=incl[:, 15:16])
            offs = smalls.tile([1, 16], f32, tag="offs")
            nc.vector.tensor_tensor(offs, incl, s_sb, Alu.subtract)
            # full cumsum: triangular prefix + broadcast chunk offsets
            cumT = psum_s.tile([128, 16], f32, tag="ps", name=f"cum{b}")
            nc.tensor.matmul(cumT, triu, gm_tm, start=True, stop=False)
            nc.tensor.matmul(cumT, ones_row, offs, start=False, stop=True)
            # negated masked centers: ct = 0.5*gm - cumsum - bigm, so the
            # attention distance is one ACT op: (pos + ct)^2 = (pos-centers)^2
            ct = ctpool.tile([128, 16], f32, tag="ct", name=f"ct{b}")
            nc.vector.scalar_tensor_tensor(
                ct, gm_tm, 0.5, cumT, Alu.mult, Alu.subtract
            )
            nc.vector.tensor_tensor(ct, ct, bigm_b[b], Alu.subtract)
            # expand to ce[p, k, l] = ct[p, k] with a zero-step read on the
            # vector engine, so the distance/exp chain runs as one wide op
            ce = cepool.tile([128, 16, L], f32, tag="ce", name=f"ce{b}")
            src_ap = bass.AP(
                tensor=ct.tensor,
                offset=ct.offset,
                ap=[list(ct.ap[0]), [1, 16], [0, L]],
            )
            nc.vector.tensor_copy(ce, src_ap)
            cts[b] = ce

        def phase_d(b, prev=None):
            """softmax attention + output matmul for batch element b.

            The attention weights are built directly in T-major layout
            ([t_chunk=128, L]) so the output matmul needs no transposes:
              ewT[t, l] = exp(-(centers[t] - pos[l])^2 / sigma2)
              out[l, c] = sum_t ewT[t, l] * xT[t, c]   (16 accumulating MMs)
              sums[l]   = sum_t ewT[t, l] * 1          (ones-column MMs)
            """
            xts = []
            for k2 in range(8):
                xt_t = xtpool.tile([128, 2, C], bf16, tag="xt")
                nc.sync.dma_start(
                    out=xt_t,
                    in_=xt[b, k2 * 256 : (k2 + 1) * 256, :].rearrange(
                        "(a p) c -> p a c", p=128
                    ),
                )
                xts.append(xt_t)
            if prev is not None:
                # previous batch's output store, emitted here so it sits
                # behind this batch's prefetches in the in-order sync queue
                pb, pob = prev
                nc.sync.dma_start(out=out[pb], in_=pob)
            ce = cts[b]
            po = psum_o.tile([L, C], f32, tag="po")
            ps = psum_s.tile([L, 1], f32, tag="ps")
            # distance and gaussian over all 16 chunks in one wide op each
            dt_ = dpool.tile([128, 16, L], f32, tag="dt", name=f"dt{b}")
            nc.vector.tensor_tensor(dt_, posK, ce, Alu.add)  # pos - centers
            nc.scalar.activation(dt_, dt_, Act.Square, bias=0.0, scale=1.0)
            ew_all = ewpool.tile([128, 16, L], bf16, tag="ewt", name=f"ew{b}")
            nc.scalar.activation(ew_all, dt_, Act.Exp, bias=0.0, scale=-1.0 / SIGMA2)
            for k in range(16):
                ewt = ew_all[:, k, :]
                nc.tensor.matmul(
                    po, ewt, xts[k // 2][:, k % 2, :], start=(k == 0), stop=(k == 15)
                )
                nc.tensor.matmul(ps, ewt, ones_col, start=(k == 0), stop=(k == 15))
            rsum = smalls.tile([L, 1], f32, tag="rsum")
            nc.vector.reciprocal(rsum, ps)
            ob = opool.tile([L, C], f32, tag="ob")
            nc.vector.tensor_scalar_mul(ob, po, rsum)
            return ob

        # conv for all batches first (dense PE stream), then the cumsum of
        # batches 0-1 overlaps the tail of conv 2-3 on the vector engine,
        # and the attention matmuls of 0-1 keep the PE busy while the
        # cumsum of 2-3 runs.
        # each phase_c(b) is emitted one conv batch late so its T-major
        # gather DMA has landed before the PE reaches its tiny matmuls
        conv_batch(0)
        conv_batch(1)
        phase_c(0)
        conv_batch(2)
        phase_c(1)
        conv_batch(3)
        phase_c(2)
        ob0 = phase_d(0)
        ob1 = phase_d(1, prev=(0, ob0))
        phase_c(3)
        ob2 = phase_d(2, prev=(1, ob1))
        ob3 = phase_d(3, prev=(2, ob2))
        # final store split across two DMA queues to shorten the tail
        nc.sync.dma_start(out=out[3, 0 : L // 2, :], in_=ob3[0 : L // 2, :])
        nc.sync.dma_start(out=out[3, L // 2 : L, :], in_=ob3[L // 2 : L, :])

    nc.compile()
    _cache["nc"] = nc
    return nc


def _in_maps(x, len_fea, conv1_w, conv1_b, conv2_w, conv2_b):
    import ml_dtypes

    bf16 = ml_dtypes.bfloat16
    f8 = ml_dtypes.float8_e4m3fn
    x = np.asarray(x, np.float32)
    x_bf = x.astype(bf16)  # [B, C, T] (feeds the output einsum, transposed)
    xt_bf = np.ascontiguousarray(x_bf.transpose(0, 2, 1))  # [B, T, C]
    # relu folded into the host prep; fp8, packed for DoubleRow:
    # [b, kg, p, i, t] with c = kg*256 + i*128 + p
    rx_bf = np.ascontiguousarray(
        np.maximum(x, 0).reshape(B, 2, 2, 128, T).transpose(0, 1, 3, 2, 4)
    ).astype(f8)
    w1t_h = np.ascontiguousarray(
        np.asarray(conv1_w, np.float32).T.reshape(2, 2, 128, C).transpose(0, 2, 1, 3)
    ).astype(f8)
    w2_h = np.ascontiguousarray(np.asarray(conv2_w, np.float32).reshape(4, 128).T).astype(bf16)
    b1_h = np.ascontiguousarray(np.asarray(conv1_b, np.float32).reshape(4, 128).T)
    b2_h = np.full((BPC, 1), float(np.asarray(conv2_b)), np.float32)
    pos_h = (0.5 + np.arange(L, dtype=np.float32)).reshape(L, 1)
    lenf_h = np.asarray(len_fea).astype(np.float32).reshape(B, 1)
    maps = []
    for i in range(NCORES):
        s = slice(i * BPC, (i + 1) * BPC)
        maps.append(
            {
                "xbf": np.ascontiguousarray(rx_bf[s]),
                "xt": np.ascontiguousarray(xt_bf[s]),
                "w1t": w1t_h,
                "w2": w2_h,
                "b1": b1_h,
                "b2r": b2_h,
                "pos": pos_h,
                "lenf": np.ascontiguousarray(lenf_h[s]),
            }
        )
    return maps


def _install_ntff_shim():
    """Provide antenv.axon_hooks (NTFF profile hook) when the image's
    antenv package lacks it, driving profiling via ctypes into
    libaxon_pjrt.so.  Needed only for BASS_TRACE=1 profiling runs."""
    import types
    import ctypes
    import contextlib

    try:
        from antenv.axon_hooks import get_axon_ntff_profile_hook  # noqa: F401

        return
    except ImportError:
        pass

    holder = {"h": None}
    mod = types.ModuleType("antenv.axon_hooks")
    mod.set_axon_ntff_profile_hook = lambda h: holder.__setitem__("h", h)
    mod.get_axon_ntff_profile_hook = lambda: holder["h"]
    sys.modules["antenv.axon_hooks"] = mod

    so_path = "/opt/axon/libaxon_pjrt.so"
    if not os.path.exists(so_path):
        return
    lib = ctypes.CDLL(so_path)
    if not hasattr(lib, "axon_start_nrt_profile"):
        return
    lib.axon_start_nrt_profile.argtypes = [
        ctypes.POINTER(ctypes.c_int64),
        ctypes.c_size_t,
    ]
    lib.axon_start_nrt_profile.restype = ctypes.c_int64
    lib.axon_stop_nrt_profile.argtypes = [ctypes.c_char_p]
    lib.axon_stop_nrt_profile.restype = ctypes.c_int64

    @contextlib.contextmanager
    def _hook(output_dir, device_ids):
        import jax

        jax.devices()
        if device_ids:
            ids = (ctypes.c_int64 * len(device_ids))(*device_ids)
            rc = lib.axon_start_nrt_profile(ids, len(device_ids))
        else:
            rc = lib.axon_start_nrt_profile(None, 0)
        if rc != 0:
            raise RuntimeError(f"axon_start_nrt_profile rc={rc}")
        try:
            yield
        finally:
            n = lib.axon_stop_nrt_profile(str(output_dir).encode())
            print(f"ntff profile: {n} file(s) written to {output_dir}", file=sys.stderr)

    mod.set_axon_ntff_profile_hook(_hook)


def kernel(x, len_fea, conv1_w, conv1_b, conv2_w, conv2_b):
    global LAST_RESULTS
    *_, bass_utils = _concourse()
    _install_ntff_shim()
    nc = _build()
    maps = _in_maps(x, len_fea, conv1_w, conv1_b, conv2_w, conv2_b)
    res = bass_utils.run_bass_kernel_spmd(nc, maps, core_ids=list(range(NCORES)))
    LAST_RESULTS = res
    feats = np.concatenate([np.asarray(r["out"]) for r in res.results], axis=0)
    lens = np.concatenate([np.asarray(r["olen"])[:, 0] for r in res.results], axis=0)
    return feats.astype(np.float32), lens.astype(np.float32)
